# revision 27
# baseline (speedup 1.0000x reference)
"""Trainium2 Bass kernel for a GPT-2-style transformer block.

B=1, T=4096, C=768, H=12 heads (hd=64), causal attention, exact GELU MLP.

Distribution over 8 NeuronCores (single shared SPMD program; collectives on
this pool measure ~0.4-1 ms per call, so the design avoids them entirely):
  - Queries: mod-8 interleaved sharding (core c owns tokens t with t%8==c),
    which makes the causal-attention instruction structure IDENTICAL on all
    cores (one shared program; per-core behavior only via input data). The
    per-core diagonal-band causal masks are fed as inputs.
  - K/V: every core computes the full-sequence K^T/V locally (replicated
    matmul — far cheaper than any collective here). Both K and V projections
    run in fp8e4m3 with DoubleRow packing (2x PE rate) off a shared fp8 copy
    of the LN output (xp8). K^T stays SBUF-resident bf16; V stays
    SBUF-resident in natural [token, feature] layout with a prepended
    ones-column per head ([P, 32, 12, 65]).
  - Attention: S^T via 2-head row-tiled matmul pairs; exp on ScalarE (no
    max-subtraction needed: |scores/8| < ~2). AV uses the 65-column V tiles:
    output partition 0 accumulates the softmax row-sum (ones column) and
    partitions 1:65 the AV product, so NO separate row-sum matmuls are
    needed. Per-pair normalization: reciprocal of the two row-sum rows, a
    2-row indicator matmul broadcasts them across partitions, two muls.
  - proj/LN2/MLP/residual: row-parallel on each core's own query rows.
  - Host: shards/transposes/casts inputs, reassembles the output.

All activations live in transposed [feature, token] layout on-chip; LN
statistics use ones-matmul partition reductions; rstd = sqrt(1/var) via
DVE reciprocal + ScalarE Sqrt (the Ln/Exp trick thrashes the activation
table: the runtime picks the first table per function, so Ln->natural_log
and Exp->exp_and_others alternate, 1.28us per load). LN1 for the K/V slabs
writes its output directly in fp8 (feeds only fp8 matmuls).

Hard-won constraints encoded here:
  - one NEFF for all 8 cores (shard_map SPMD) -> no per-core control flow;
  - PSUM is 8 banks: attention runs s(2x2)+y0(1)+y1(1)+bc(2),
    MLP fc1(2)+fc2(6);
  - fp8 V/K error diffuses through softmax and the ~1%-of-output attention
    branch; MLP must stay bf16 (its output is ~30% of the residual, fp8
    there costs ~1.6e-2 rel err).
"""

import numpy as np
import ml_dtypes

import concourse.bacc as bacc
import concourse.mybir as mybir
import concourse.tile as tile
from concourse.bass_utils import run_bass_kernel_spmd

BF16 = ml_dtypes.bfloat16
F8 = ml_dtypes.float8_e4m3

# problem shape (hardcoded per harness contract)
T = 4096
C = 768
H = 12
HD = 64
EPS = 1e-5
NC = 8          # cores
R = 512         # tokens per core
P = 128
CT = C // P     # 6 feature tiles
QT = R // P     # 4 query tiles per core
KT = T // P     # 32 key tiles
PAIRS = H // 2  # 6 head pairs
HT = (4 * C) // P  # 24 hidden tiles

_CACHE = {}


def _ln_transposed(nc, tc, pool, pspool, xT, out_bf, ones_sb, w_col, b_col, apply_wb,
                   x_is_bf16=False, alt=0):
    """LayerNorm over the feature axis for [C, R]-transposed activations.

    xT: f32 (or bf16 with x_is_bf16) sbuf tile [P, CT, R]; out_bf: bf16 or
    fp8 tile. Stats via ones-matmul partition reduction (all-partition-
    broadcast results), rstd = Sqrt(reciprocal(var+eps)).
    """
    f32 = mybir.dt.float32
    bf16 = mybir.dt.bfloat16
    if x_is_bf16:
        xb = xT
    else:
        xb = pool.tile([P, CT, R], bf16, tag="ln_xb")
    sq = pool.tile([P, CT, R], bf16, tag="ln_sq")
    for k in range(CT):
        eng = nc.vector if (k + alt) % 2 == 0 else nc.gpsimd
        if not x_is_bf16:
            eng.tensor_copy(xb[:, k, :], xT[:, k, :])
        eng.tensor_mul(sq[:, k, :], xb[:, k, :], xb[:, k, :])
    ps_sum = pspool.tile([P, R], f32, tag="ln_psum")
    ps_sq = pspool.tile([P, R], f32, tag="ln_pssq")
    for k in range(CT):
        nc.tensor.matmul(ps_sum[:], ones_sb[:], xb[:, k, :], start=(k == 0), stop=(k == CT - 1))
    for k in range(CT):
        nc.tensor.matmul(ps_sq[:], ones_sb[:], sq[:, k, :], start=(k == 0), stop=(k == CT - 1))
    nmean = pool.tile([P, R], f32, tag="ln_nmean")
    m2 = pool.tile([P, R], f32, tag="ln_m2")
    sm = nc.vector if alt == 0 else nc.gpsimd
    sm.tensor_scalar_mul(nmean[:], ps_sum[:], -1.0 / C)
    sm.tensor_scalar_mul(m2[:], ps_sq[:], 1.0 / C)
    var = pool.tile([P, R], f32, tag="ln_var")
    sm.tensor_mul(var[:], nmean[:], nmean[:])          # mean^2
    # var = (E[x^2] + eps) - mean^2
    sm.scalar_tensor_tensor(
        var[:], m2[:], EPS, var[:], mybir.AluOpType.add, mybir.AluOpType.subtract
    )
    rstd = pool.tile([P, R], f32, tag="ln_rstd")
    nc.vector.reciprocal(rstd[:], var[:])
    nc.scalar.activation(rstd[:], rstd[:], mybir.ActivationFunctionType.Sqrt)
    nmr = pool.tile([P, R], f32, tag="ln_nmr")
    sm.tensor_mul(nmr[:], nmean[:], rstd[:])           # -mu*rstd
    tmp = pool.tile([P, R], f32, tag="ln_tmp")
    tmp2 = pool.tile([P, R], f32, tag="ln_tmp2")
    for k in range(CT):
        eng = nc.vector if (k + alt) % 2 == 0 else nc.gpsimd
        t = tmp if (k + alt) % 2 == 0 else tmp2
        eng.tensor_mul(t[:], xT[:, k, :], rstd[:])
        if apply_wb:
            eng.tensor_add(t[:], t[:], nmr[:])
            eng.tensor_scalar(
                out_bf[:, k, :], t[:], w_col[:, k : k + 1], b_col[:, k : k + 1],
                mybir.AluOpType.mult, mybir.AluOpType.add,
            )
        else:
            eng.tensor_add(out_bf[:, k, :], t[:], nmr[:])


def _build(apply_ln1, apply_ln2, apply_bv, apply_bqk=False, sim_no_cc=False, reps=1):
    key = (apply_ln1, apply_ln2, apply_bv, apply_bqk, reps)
    if key in _CACHE:
        return _CACHE[key]

    f32 = mybir.dt.float32
    bf16 = mybir.dt.bfloat16
    f8 = mybir.dt.float8e4
    AF = mybir.ActivationFunctionType

    nc = bacc.Bacc("TRN2", target_bir_lowering=False, debug=False, num_devices=NC)

    xtf_d = nc.declare_dram_parameter("xtf", [C, T], bf16, isOutput=False)
    xTq_d = nc.declare_dram_parameter("xTq", [C, R], f32, isOutput=False)
    masks_d = nc.declare_dram_parameter("masks", [P, 8, P], bf16, isOutput=False)
    ones_d = nc.declare_dram_parameter("ones", [P, P], bf16, isOutput=False)
    ind_d = nc.declare_dram_parameter("ind", [2, P], bf16, isOutput=False)
    wattn_d = nc.declare_dram_parameter("wattn", [C, C], bf16, isOutput=False)  # Q cols
    wk8_d = nc.declare_dram_parameter("wk8", [HD, CT, 2, C], f8, isOutput=False)
    wv8_d = nc.declare_dram_parameter("wv8", [HD, CT, 2, C], f8, isOutput=False)
    wproj_d = nc.declare_dram_parameter("wproj", [C, C], bf16, isOutput=False)
    wfc_d = nc.declare_dram_parameter("wfc", [C, 4 * C], bf16, isOutput=False)
    wfc2_d = nc.declare_dram_parameter("wfc2", [4 * C, C], bf16, isOutput=False)
    bqk_d = nc.declare_dram_parameter("bqk", [P, 2 * CT], f32, isOutput=False)
    bproj_d = nc.declare_dram_parameter("bproj", [P, CT], f32, isOutput=False)
    bfc_d = nc.declare_dram_parameter("bfc", [P, HT], f32, isOutput=False)
    bfc2_d = nc.declare_dram_parameter("bfc2", [P, CT], f32, isOutput=False)
    if apply_bv:
        bv_d = nc.declare_dram_parameter("bv", [P, C], f32, isOutput=False)
    if apply_ln1:
        ln1w_d = nc.declare_dram_parameter("ln1w", [P, CT], f32, isOutput=False)
        ln1b_d = nc.declare_dram_parameter("ln1b", [P, CT], f32, isOutput=False)
    if apply_ln2:
        ln2w_d = nc.declare_dram_parameter("ln2w", [P, CT], f32, isOutput=False)
        ln2b_d = nc.declare_dram_parameter("ln2b", [P, CT], f32, isOutput=False)
    outT_d = nc.declare_dram_parameter("outT", [C, R], f32, isOutput=True)

    with tile.TileContext(nc) as tc:
        with (
            tc.tile_pool(name="const", bufs=1) as const,
            tc.tile_pool(name="mid", bufs=1) as mid,
        ):
            ones_sb = const.tile([P, P], bf16)
            nc.sync.dma_start(ones_sb[:], ones_d[:])
            ind_sb = const.tile([2, P], bf16)
            nc.sync.dma_start(ind_sb[:], ind_d[:])
            masks_sb = const.tile([P, 8, P], bf16)
            nc.gpsimd.dma_start(masks_sb[:], masks_d[:])
            bqk_sb = const.tile([P, 2 * CT], f32)
            nc.sync.dma_start(bqk_sb[:], bqk_d[:])
            bproj_sb = const.tile([P, CT], f32)
            nc.gpsimd.dma_start(bproj_sb[:], bproj_d[:])
            bfc_sb = const.tile([P, HT], f32)
            nc.gpsimd.dma_start(bfc_sb[:], bfc_d[:])
            bfc2_sb = const.tile([P, CT], f32)
            nc.gpsimd.dma_start(bfc2_sb[:], bfc2_d[:])
            if apply_bv:
                bv_sb = const.tile([P, C], f32)
                nc.sync.dma_start(bv_sb[:], bv_d[:])
            ln1w_sb = ln1b_sb = ln2w_sb = ln2b_sb = None
            if apply_ln1:
                ln1w_sb = const.tile([P, CT], f32)
                ln1b_sb = const.tile([P, CT], f32)
                nc.sync.dma_start(ln1w_sb[:], ln1w_d[:])
                nc.sync.dma_start(ln1b_sb[:], ln1b_d[:])
            if apply_ln2:
                ln2w_sb = const.tile([P, CT], f32)
                ln2b_sb = const.tile([P, CT], f32)
                nc.sync.dma_start(ln2w_sb[:], ln2w_d[:])
                nc.sync.dma_start(ln2b_sb[:], ln2b_d[:])
            xTq_sb = const.tile([P, CT, R], f32)
            nc.sync.dma_start(xTq_sb[:], xTq_d.rearrange("(o p) t -> p o t", p=P))

            # mid-lifetime tiles
            # Q^T, fp8 DoubleRow-packed along hd for the S matmuls:
            # head h lives on partitions 32*(h%3) : +32 (matmul operands must
            # base at 0/32/64), group g=h//3, hd split as 32*j + p (j free).
            q8 = mid.tile([P, H // 3, 2, R], f8)
            ynorm_sb = mid.tile([P, CT, R], bf16)  # normalized attn out (y^T)
            # V resident, natural [token, feature] layout, 65 cols per head:
            # col 0 = ones (row-sum accumulator lane), cols 1:65 = V features.
            v_res = mid.tile([P, KT, H, 65], bf16)

            for _rep in range(reps):
                # ---------------- Phase A: LN1 + Q + full K/V ----------------
                ktp_cm = tc.tile_pool(name="ktp", bufs=1)
                ktp = ktp_cm.__enter__()
                # resident K^T, fp8 DoubleRow-packed along hd (same head
                # placement as q8)
                kt8 = ktp.tile([P, H // 3, 2, T], f8)
                with (
                    tc.tile_pool(name="qkvp", bufs=1) as qkvp,
                    tc.tile_pool(name="lnp", bufs=1) as lnp,
                    tc.tile_pool(name="chkp", bufs=2) as chkp,
                    tc.tile_pool(name="ps_ln", bufs=1, space="PSUM") as ps_ln,
                    tc.tile_pool(name="ps_qkv", bufs=3, space="PSUM") as ps_qkv,
                ):
                    # ones column of v_res (written once, read by every AV matmul)
                    nc.gpsimd.tensor_copy(
                        v_res[:, :, :, 0:1],
                        ones_sb[:, 0:1].to_broadcast((P, KT, H, 1)),
                    )
                    wattn_sb = qkvp.tile([P, CT, C], bf16)
                    wk8_sb = qkvp.tile([HD, CT, 2, C], f8)
                    nc.gpsimd.dma_start(wk8_sb[:], wk8_d[:])
                    wv8_sb = qkvp.tile([HD, CT, 2, C], f8)
                    nc.gpsimd.dma_start(wv8_sb[:], wv8_d[:])
                    wattn_r = wattn_d.rearrange("(o p) f -> p o f", p=P)
                    for k in range(CT):
                        nc.sync.dma_start(wattn_sb[:, k, :], wattn_r[:, k, :])

                    # Q^T for own (interleaved) query rows — first, so attention
                    # can begin as soon as the early K/V slabs land.
                    # wattn/wk8 columns are host-permuted so psum partition
                    # p^ = 64j + 32e + q holds feature 64e + 32j + q of the
                    # head pair: the two j-halves are then contiguous 64-
                    # partition blocks, repacked to DoubleRow with 2 DMAs.
                    xln_q = qkvp.tile([P, CT, R], bf16)
                    _ln_transposed(nc, tc, lnp, ps_ln, xTq_sb, xln_q, ones_sb,
                                   ln1w_sb, ln1b_sb, apply_ln1)
                    for f in range(CT):
                        ps = ps_qkv.tile([P, R], f32, tag="qk_ps")
                        for k in range(CT):
                            nc.tensor.matmul(
                                ps[:], wattn_sb[:, k, P * f : P * (f + 1)],
                                xln_q[:, k, :], start=(k == 0), stop=(k == CT - 1),
                            )
                        qst = chkp.tile([P, R], f8, tag="qst", name="qst")
                        nc.vector.tensor_scalar(
                            qst[:], ps[:], bqk_sb[:, f : f + 1], None,
                            mybir.AluOpType.add,
                        )
                        for e in range(2):
                            h = 2 * f + e
                            for j in range(2):
                                nc.gpsimd.dma_start(
                                    q8[32 * (h % 3) : 32 * (h % 3) + 32, h // 3, j, :],
                                    qst[64 * j + 32 * e : 64 * j + 32 * e + 32, :],
                                )

                    xtf_r = xtf_d.rearrange("(o p) t -> p o t", p=P)
                    for s in range(NC):
                        xv = chkp.tile([P, CT, R], bf16, tag="xv", name="xv")
                        xeng = nc.sync if s % 2 == 0 else nc.gpsimd
                        xeng.dma_start(xv[:], xtf_r[:, :, R * s : R * (s + 1)])
                        # LN output written directly as fp8 (feeds only the
                        # fp8 K/V DoubleRow matmuls)
                        xln8 = chkp.tile([P, CT, R], f8, tag="xln8", name="xln8")
                        _ln_transposed(nc, tc, lnp, ps_ln, xv, xln8, ones_sb,
                                       ln1w_sb, ln1b_sb, apply_ln1, x_is_bf16=True)
                        # repack to DoubleRow layout [64, CT, 2, R]:
                        # contraction row c = j*64 + ki.
                        xp8 = chkp.tile([HD, CT, 2, R], f8, tag="xp8", name="xp8")
                        nc.gpsimd.dma_start(xp8[:, :, 0, :], xln8[0:HD, :, :])
                        nc.gpsimd.dma_start(xp8[:, :, 1, :], xln8[HD:P, :, :])

                        # K^T slab -> fp8 stage -> DoubleRow repack DMAs
                        for f in range(CT):
                            ps = ps_qkv.tile([P, R], f32, tag="qk_ps")
                            for k in range(CT):
                                nc.tensor.matmul(
                                    ps[:], wk8_sb[:, k, :, P * f : P * (f + 1)],
                                    xp8[:, k, :, :], start=(k == 0), stop=(k == CT - 1),
                                    perf_mode=mybir.MatmulPerfMode.DoubleRow,
                                )
                            kst = chkp.tile([P, R], f8, tag="kst", name="kst")
                            keng = nc.vector if (f + s) % 2 == 0 else nc.gpsimd
                            if apply_bqk:
                                keng.tensor_scalar(
                                    kst[:], ps[:],
                                    bqk_sb[:, CT + f : CT + f + 1], None,
                                    mybir.AluOpType.add,
                                )
                            else:
                                keng.tensor_copy(kst[:], ps[:])
                            deng = nc.sync if (f + s) % 2 == 0 else nc.gpsimd
                            for e in range(2):
                                h = 2 * f + e
                                for j in range(2):
                                    deng.dma_start(
                                        kt8[32 * (h % 3) : 32 * (h % 3) + 32, h // 3, j,
                                            R * s : R * (s + 1)],
                                        kst[64 * j + 32 * e : 64 * j + 32 * e + 32, :],
                                    )

                        # V slab -> natural [token, feature] layout, fp8
                        # DoubleRow matmuls, evac strided into the 65-col
                        # per-head groups of v_res
                        for t in range(QT):
                            kt = QT * s + t
                            for hh in range(2):
                                ps = ps_qkv.tile([P, 384], f32, tag="v_ps")
                                for k in range(CT):
                                    nc.tensor.matmul(
                                        ps[:], xp8[:, k, :, P * t : P * (t + 1)],
                                        wv8_sb[:, k, :, 384 * hh : 384 * (hh + 1)],
                                        start=(k == 0), stop=(k == CT - 1),
                                        perf_mode=mybir.MatmulPerfMode.DoubleRow,
                                    )
                                dst = v_res[:, kt, 6 * hh : 6 * (hh + 1), 1:65]
                                psv = ps[:].rearrange("p (h f) -> p h f", h=6)
                                veng = nc.vector if (t + hh) % 2 == 0 else nc.gpsimd
                                if apply_bv:
                                    bvv = bv_sb[:, 384 * hh : 384 * (hh + 1)].rearrange(
                                        "p (h f) -> p h f", h=6
                                    )
                                    veng.tensor_add(dst, psv, bvv)
                                else:
                                    veng.tensor_copy(dst, psv)

                # ---------------- Phase B: attention ----------------
                with (
                    tc.tile_pool(name="pp", bufs=2) as pp,
                    tc.tile_pool(name="normp", bufs=2) as normp,
                    tc.tile_pool(name="ps_s", bufs=2, space="PSUM") as ps_s,
                    tc.tile_pool(name="ps_y0", bufs=1, space="PSUM") as ps_y0,
                    tc.tile_pool(name="ps_y1", bufs=1, space="PSUM") as ps_y1,
                    tc.tile_pool(name="ps_bc", bufs=2, space="PSUM") as ps_bc,
                ):
                    for pr in range(PAIRS):
                        y0_ps = ps_y0.tile([P, R], f32, tag="y0")
                        y1_ps = ps_y1.tile([P, R], f32, tag="y1")
                        for m in range(4):  # bands of 8 key-tiles
                            N = P * (4 - m)
                            # p_band is h-major: [P, head-of-pair, ktile-in-band, R]
                            p_band = pp.tile([P, 2, 8, R], bf16, tag="p")
                            # exp-group size: G*N == 512 for bands 2-3, so each
                            # h-plane of the [P, 2, 512] psum tile (1 bank each,
                            # bank-aligned) holds G k-tiles' scores.
                            G = (1, 1, 2, 4)[m]
                            for g in range(8 // G):
                                s_ps = ps_s.tile([P, 2, R], f32, tag="s", name="s_ps")
                                for dg in range(G):
                                    d = g * G + dg
                                    k = 8 * m + d
                                    for e in range(2):
                                        h = 2 * pr + e
                                        hb = 32 * (h % 3)
                                        hg = h // 3
                                        nc.tensor.matmul(
                                            s_ps[:, e, dg * N : (dg + 1) * N],
                                            kt8[hb : hb + 32, hg, :, P * k : P * (k + 1)],
                                            q8[hb : hb + 32, hg, :, 0:N],
                                            perf_mode=mybir.MatmulPerfMode.DoubleRow,
                                            skip_group_check=True,
                                        )
                                if G == 1:
                                    nc.scalar.activation(
                                        p_band[:, :, g, :N], s_ps[:, :, :N],
                                        AF.Exp, scale=0.125,
                                    )
                                else:
                                    nc.scalar.activation(
                                        p_band[:, :, g * G : (g + 1) * G, :N],
                                        s_ps[:].rearrange("p h (a n) -> p h a n", n=N),
                                        AF.Exp, scale=0.125,
                                    )
                            for d in range(8):
                                k = 8 * m + d
                                # causal mask on this k-tile's diagonal col-group
                                meng = nc.gpsimd if d % 3 == 1 else nc.vector
                                meng.tensor_mul(
                                    p_band[:, :, d, N - P : N],
                                    p_band[:, :, d, N - P : N],
                                    masks_sb[:, d : d + 1, :].to_broadcast((P, 2, P)),
                                )
                                # AV + row-sum in one matmul per head: V's
                                # ones-column accumulates the row-sum into
                                # output partition 0
                                nc.tensor.matmul(
                                    y0_ps[0:65, 0:N], v_res[:, k, 2 * pr, :],
                                    p_band[:, 0, d, :N],
                                    start=(k == 0), stop=(k == KT - 1),
                                    skip_group_check=True,
                                )
                                nc.tensor.matmul(
                                    y1_ps[0:65, 0:N], v_res[:, k, 2 * pr + 1, :],
                                    p_band[:, 1, d, :N],
                                    start=(k == 0), stop=(k == KT - 1),
                                    skip_group_check=True,
                                )
                        # normalization: recip of the two row-sum rows, then
                        # 1-row ones-matmuls broadcast them across partitions
                        recip0 = normp.tile([1, R], bf16, tag="recip0")
                        recip1 = normp.tile([1, R], bf16, tag="recip1")
                        with nc.allow_low_precision(reason="softmax denom recip to bf16"):
                            nc.vector.reciprocal(recip0[:], y0_ps[0:1, :])
                            nc.vector.reciprocal(recip1[:], y1_ps[0:1, :])
                        bc_ps = ps_bc.tile([P, R], f32, tag="bc")
                        nc.tensor.matmul(bc_ps[0:HD, :], ones_sb[0:1, 0:HD], recip0[:],
                                         skip_group_check=True)
                        nc.tensor.matmul(bc_ps[HD:P, :], ones_sb[0:1, 0:HD], recip1[:],
                                         skip_group_check=True)
                        nc.vector.tensor_mul(
                            ynorm_sb[0:HD, pr, :], y0_ps[1:65, :], bc_ps[0:HD, :]
                        )
                        nc.vector.tensor_mul(
                            ynorm_sb[HD:P, pr, :], y1_ps[1:65, :], bc_ps[HD:P, :]
                        )

                ktp_cm.__exit__(None, None, None)

                # ---------------- Phase C: proj + LN2 + MLP + out ----------------
                with (
                    tc.tile_pool(name="mlpp", bufs=1) as mlpp,
                    tc.tile_pool(name="lnp2", bufs=1) as lnp2,
                ):
                    # weight loads split across queues (sync/gpsimd/vector/
                    # scalar) so no single 14us DMA gates the MLP
                    wproj_sb = mlpp.tile([P, CT, C], bf16)
                    wproj_r = wproj_d.rearrange("(o p) f -> p o f", p=P)
                    nc.sync.dma_start(wproj_sb[:, 0:3, :], wproj_r[:, 0:3, :])
                    nc.gpsimd.dma_start(wproj_sb[:, 3:6, :], wproj_r[:, 3:6, :])
                    wfc_sb = mlpp.tile([P, CT, 4 * C], bf16)
                    wfc_r = wfc_d.rearrange("(o p) f -> p o f", p=P)
                    dengs = [nc.sync, nc.gpsimd, nc.scalar, nc.gpsimd]
                    for ch in range(4):
                        dengs[ch % 4].dma_start(
                            wfc_sb[:, :, C * ch : C * (ch + 1)],
                            wfc_r[:, :, C * ch : C * (ch + 1)],
                        )
                    z_sb = mlpp.tile([P, CT, R], f32)      # residual stream x+attn
                    xln2_sb = mlpp.tile([P, CT, R], bf16)
                    wfc2_sb = mlpp.tile([P, HT, C], bf16)
                    wfc2_r = wfc2_d.rearrange("(o p) f -> p o f", p=P)
                    for ch in range(4):
                        dengs[ch % 4].dma_start(
                            wfc2_sb[:, CT * ch : CT * (ch + 1), :],
                            wfc2_r[:, CT * ch : CT * (ch + 1), :],
                        )
                    with (
                        tc.tile_pool(name="ps_proj", bufs=2, space="PSUM") as ps_proj,
                        tc.tile_pool(name="ps_ln2", bufs=1, space="PSUM") as ps_ln2,
                    ):
                        for f in range(CT):
                            ps = ps_proj.tile([P, R], f32, tag="proj")
                            for k in range(CT):
                                nc.tensor.matmul(
                                    ps[:], wproj_sb[:, k, P * f : P * (f + 1)],
                                    ynorm_sb[:, k, :], start=(k == 0), stop=(k == CT - 1),
                                )
                            # z = (proj + b_proj) + x
                            nc.vector.scalar_tensor_tensor(
                                z_sb[:, f, :], ps[:], bproj_sb[:, f : f + 1], xTq_sb[:, f, :],
                                mybir.AluOpType.add, mybir.AluOpType.add,
                            )
                        _ln_transposed(nc, tc, lnp2, ps_ln2, z_sb, xln2_sb, ones_sb,
                                       ln2w_sb, ln2b_sb, apply_ln2)

                    h_sb = mlpp.tile([P, CT, R], bf16)
                    with (
                        tc.tile_pool(name="ps_fc1", bufs=2, space="PSUM") as ps_fc1,
                        tc.tile_pool(name="ps_o", bufs=1, space="PSUM") as ps_o,
                    ):
                        o_ps = [ps_o.tile([P, R], f32, tag=f"o{f}", name=f"o_ps{f}") for f in range(CT)]
                        for chunk in range(4):
                            for hf in range(CT):
                                hh = CT * chunk + hf
                                ps = ps_fc1.tile([P, R], f32, tag="fc1")
                                for k in range(CT):
                                    nc.tensor.matmul(
                                        ps[:], wfc_sb[:, k, P * hh : P * (hh + 1)],
                                        xln2_sb[:, k, :], start=(k == 0), stop=(k == CT - 1),
                                    )
                                nc.scalar.activation(
                                    h_sb[:, hf, :], ps[:], AF.Gelu, bias=bfc_sb[:, hh : hh + 1]
                                )
                                for f in range(CT):
                                    nc.tensor.matmul(
                                        o_ps[f][:], wfc2_sb[:, hh, P * f : P * (f + 1)],
                                        h_sb[:, hf, :], start=(hh == 0), stop=(hh == HT - 1),
                                    )
                        for f in range(CT):
                            # out = (o + b_fc2) + z, in place over z
                            nc.vector.scalar_tensor_tensor(
                                z_sb[:, f, :], o_ps[f][:], bfc2_sb[:, f : f + 1], z_sb[:, f, :],
                                mybir.AluOpType.add, mybir.AluOpType.add,
                            )
                            nc.sync.dma_start(outT_d[P * f : P * (f + 1), :], z_sb[:, f, :])

    nc.compile()
    _CACHE[key] = nc
    return nc


def _query_tokens(c):
    """Token ids owned by core c, in on-chip column order (j desc, i asc)."""
    return np.concatenate([1024 * j + 8 * np.arange(P) + c for j in (3, 2, 1, 0)])


def kernel(x, ln1_w, ln1_b, W_attn, b_attn, W_proj, b_proj,
           ln2_w, ln2_b, W_fc, b_fc, W_fc2, b_fc2):
    x = np.asarray(x, np.float32)
    ln1_w = np.asarray(ln1_w, np.float32)
    ln1_b = np.asarray(ln1_b, np.float32)
    W_attn = np.asarray(W_attn, np.float32)
    b_attn = np.asarray(b_attn, np.float32)
    W_proj = np.asarray(W_proj, np.float32)
    b_proj = np.asarray(b_proj, np.float32)
    ln2_w = np.asarray(ln2_w, np.float32)
    ln2_b = np.asarray(ln2_b, np.float32)
    W_fc = np.asarray(W_fc, np.float32)
    b_fc = np.asarray(b_fc, np.float32)
    W_fc2 = np.asarray(W_fc2, np.float32)
    b_fc2 = np.asarray(b_fc2, np.float32)

    apply_ln1 = not (np.all(ln1_w == 1.0) and np.all(ln1_b == 0.0))
    apply_ln2 = not (np.all(ln2_w == 1.0) and np.all(ln2_b == 0.0))
    apply_bv = bool(np.any(b_attn[2 * C :] != 0.0))
    apply_bqk = bool(np.any(b_attn[: 2 * C] != 0.0))

    nc = _build(apply_ln1, apply_ln2, apply_bv, apply_bqk)

    xf = x[0]  # [T, C]
    # Q/K psum partition permutation: psum partition p^ = 64j + 32e + q
    # holds pair-local feature 64e + 32j + q (e = head-in-pair, hd = 32j+q),
    # so the two j-halves are contiguous 64-partition blocks for the
    # DoubleRow repack DMAs.
    ph = np.arange(P)
    PERM = 64 * ((ph % 64) // 32) + 32 * (ph // 64) + ph % 32
    wq = W_attn[:, :C].reshape(C, CT, P)[:, :, PERM].reshape(C, C)
    wattn_b = np.ascontiguousarray(wq).astype(BF16)
    # DoubleRow packing: [ki, kk, j, f] = W[kk*128 + j*64 + ki, col0 + f]
    wkp = W_attn[:, C : 2 * C].reshape(C, CT, P)[:, :, PERM].reshape(C, C)
    wk = wkp.reshape(CT, 2, HD, C)  # [kk, j, ki, f]
    wk8 = np.ascontiguousarray(wk.transpose(2, 0, 1, 3)).astype(F8)
    wv = W_attn[:, 2 * C :].reshape(CT, 2, HD, C)
    wv8 = np.ascontiguousarray(wv.transpose(2, 0, 1, 3)).astype(F8)
    wproj_b = W_proj.astype(BF16)
    wfc_b = W_fc.astype(BF16)
    wfc2_b = W_fc2.astype(BF16)
    bqk = np.ascontiguousarray(b_attn[: 2 * C].reshape(2 * CT, P)[:, PERM].T)
    bproj = np.ascontiguousarray(b_proj.reshape(CT, P).T)
    bfc = np.ascontiguousarray(b_fc.reshape(HT, P).T)
    bfc2 = np.ascontiguousarray(b_fc2.reshape(CT, P).T)
    ones = np.ones((P, P), BF16)
    ind = np.zeros((2, P), BF16)
    ind[0, 0:HD] = 1.0
    ind[1, HD:P] = 1.0

    xtf = np.ascontiguousarray(xf.T.astype(BF16))
    in_maps = []
    qtok = []
    for c in range(NC):
        qt = _query_tokens(c)
        qtok.append(qt)
        xTq = np.ascontiguousarray(xf[qt, :].T)
        kk = np.arange(P)[:, None, None]
        dd = np.arange(8)[None, :, None]
        ii = np.arange(P)[None, None, :]
        masks = ((8 * ii + c - 128 * dd - kk) >= 0).astype(BF16)
        m = {
            "xtf": xtf, "xTq": xTq, "masks": masks, "ones": ones, "ind": ind,
            "wk8": wk8, "wv8": wv8,
            "wattn": wattn_b, "wproj": wproj_b, "wfc": wfc_b, "wfc2": wfc2_b,
            "bqk": bqk, "bproj": bproj, "bfc": bfc, "bfc2": bfc2,
        }
        if apply_bv:
            m["bv"] = np.ascontiguousarray(np.broadcast_to(b_attn[2 * C :], (P, C)))
        if apply_ln1:
            m["ln1w"] = np.ascontiguousarray(ln1_w.reshape(CT, P).T)
            m["ln1b"] = np.ascontiguousarray(ln1_b.reshape(CT, P).T)
        if apply_ln2:
            m["ln2w"] = np.ascontiguousarray(ln2_w.reshape(CT, P).T)
            m["ln2b"] = np.ascontiguousarray(ln2_b.reshape(CT, P).T)
        in_maps.append(m)

    res = run_bass_kernel_spmd(nc, in_maps, list(range(NC)))

    out = np.empty((T, C), np.float32)
    for c in range(NC):
        out[qtok[c], :] = res.results[c]["outT"].T
    return out[None, :, :]


# revision 31
# speedup vs baseline: 1.0262x; 1.0262x over previous
"""Trainium2 Bass kernel for a GPT-2-style transformer block.

B=1, T=4096, C=768, H=12 heads (hd=64), causal attention, exact GELU MLP.

Distribution over 8 NeuronCores (single shared SPMD program; collectives on
this pool measure ~0.4-1 ms per call, so the design avoids them entirely):
  - Queries: mod-8 interleaved sharding (core c owns tokens t with t%8==c),
    which makes the causal-attention instruction structure IDENTICAL on all
    cores (one shared program; per-core behavior only via input data). The
    per-core diagonal-band causal masks are fed as inputs.
  - K/V: every core computes the full-sequence K^T/V locally (replicated
    matmul — far cheaper than any collective here). Both K and V projections
    run in fp8e4m3 with DoubleRow packing (2x PE rate) off a shared fp8 copy
    of the LN output (xp8). K^T stays SBUF-resident bf16; V stays
    SBUF-resident in natural [token, feature] layout with a prepended
    ones-column per head ([P, 32, 12, 65]).
  - Attention: S^T via 2-head row-tiled matmul pairs; exp on ScalarE (no
    max-subtraction needed: |scores/8| < ~2). AV uses the 65-column V tiles:
    output partition 0 accumulates the softmax row-sum (ones column) and
    partitions 1:65 the AV product, so NO separate row-sum matmuls are
    needed. Per-pair normalization: reciprocal of the two row-sum rows, a
    2-row indicator matmul broadcasts them across partitions, two muls.
  - proj/LN2/MLP/residual: row-parallel on each core's own query rows.
  - Host: shards/transposes/casts inputs, reassembles the output.

All activations live in transposed [feature, token] layout on-chip; LN
statistics use ones-matmul partition reductions; rstd = sqrt(1/var) via
DVE reciprocal + ScalarE Sqrt (the Ln/Exp trick thrashes the activation
table: the runtime picks the first table per function, so Ln->natural_log
and Exp->exp_and_others alternate, 1.28us per load). LN1 for the K/V slabs
writes its output directly in fp8 (feeds only fp8 matmuls).

Hard-won constraints encoded here:
  - one NEFF for all 8 cores (shard_map SPMD) -> no per-core control flow;
  - PSUM is 8 banks: attention runs s(2x2)+y0(1)+y1(1)+bc(2),
    MLP fc1(2)+fc2(6);
  - fp8 V/K error diffuses through softmax and the ~1%-of-output attention
    branch; MLP must stay bf16 (its output is ~30% of the residual, fp8
    there costs ~1.6e-2 rel err).
"""

import numpy as np
import ml_dtypes

import concourse.bacc as bacc
import concourse.mybir as mybir
import concourse.tile as tile
from concourse.bass_utils import run_bass_kernel_spmd

BF16 = ml_dtypes.bfloat16
F8 = ml_dtypes.float8_e4m3

# problem shape (hardcoded per harness contract)
T = 4096
C = 768
H = 12
HD = 64
EPS = 1e-5
NC = 8          # cores
R = 512         # tokens per core
P = 128
CT = C // P     # 6 feature tiles
QT = R // P     # 4 query tiles per core
KT = T // P     # 32 key tiles
PAIRS = H // 2  # 6 head pairs
HT = (4 * C) // P  # 24 hidden tiles

_CACHE = {}


def _ln_transposed(nc, tc, pool, pspool, xT, out_bf, ones_sb, w_col, b_col, apply_wb,
                   x_is_bf16=False, alt=0):
    """LayerNorm over the feature axis for [C, R]-transposed activations.

    xT: f32 (or bf16 with x_is_bf16) sbuf tile [P, CT, R]; out_bf: bf16 or
    fp8 tile. Stats via ones-matmul partition reduction (all-partition-
    broadcast results), rstd = Sqrt(reciprocal(var+eps)).
    """
    f32 = mybir.dt.float32
    bf16 = mybir.dt.bfloat16
    if x_is_bf16:
        xb = xT
    else:
        xb = pool.tile([P, CT, R], bf16, tag="ln_xb")
    sq = pool.tile([P, CT, R], bf16, tag="ln_sq")
    for k in range(CT):
        eng = nc.gpsimd if (k + alt) % 3 == 2 else nc.vector
        if not x_is_bf16:
            eng.tensor_copy(xb[:, k, :], xT[:, k, :])
        eng.tensor_mul(sq[:, k, :], xb[:, k, :], xb[:, k, :])
    ps_sum = pspool.tile([P, R], f32, tag="ln_psum")
    ps_sq = pspool.tile([P, R], f32, tag="ln_pssq")
    for k in range(CT):
        nc.tensor.matmul(ps_sum[:], ones_sb[:], xb[:, k, :], start=(k == 0), stop=(k == CT - 1))
    for k in range(CT):
        nc.tensor.matmul(ps_sq[:], ones_sb[:], sq[:, k, :], start=(k == 0), stop=(k == CT - 1))
    nmean = pool.tile([P, R], f32, tag="ln_nmean")
    m2 = pool.tile([P, R], f32, tag="ln_m2")
    sm = nc.vector if alt == 0 else nc.gpsimd
    sm.tensor_scalar_mul(nmean[:], ps_sum[:], -1.0 / C)
    sm.tensor_scalar_mul(m2[:], ps_sq[:], 1.0 / C)
    var = pool.tile([P, R], f32, tag="ln_var")
    sm.tensor_mul(var[:], nmean[:], nmean[:])          # mean^2
    # var = (E[x^2] + eps) - mean^2
    sm.scalar_tensor_tensor(
        var[:], m2[:], EPS, var[:], mybir.AluOpType.add, mybir.AluOpType.subtract
    )
    rstd = pool.tile([P, R], f32, tag="ln_rstd")
    nc.vector.reciprocal(rstd[:], var[:])
    nc.scalar.activation(rstd[:], rstd[:], mybir.ActivationFunctionType.Sqrt)
    nmr = pool.tile([P, R], f32, tag="ln_nmr")
    sm.tensor_mul(nmr[:], nmean[:], rstd[:])           # -mu*rstd
    tmp = pool.tile([P, R], f32, tag="ln_tmp")
    tmp2 = pool.tile([P, R], f32, tag="ln_tmp2")
    for k in range(CT):
        eng = nc.gpsimd if (k + alt) % 3 == 2 else nc.vector
        t = tmp2 if (k + alt) % 3 == 2 else tmp
        eng.tensor_mul(t[:], xT[:, k, :], rstd[:])
        if apply_wb:
            eng.tensor_add(t[:], t[:], nmr[:])
            eng.tensor_scalar(
                out_bf[:, k, :], t[:], w_col[:, k : k + 1], b_col[:, k : k + 1],
                mybir.AluOpType.mult, mybir.AluOpType.add,
            )
        else:
            eng.tensor_add(out_bf[:, k, :], t[:], nmr[:])


def _build(apply_ln1, apply_ln2, apply_bv, apply_bqk=False, sim_no_cc=False, reps=1):
    key = (apply_ln1, apply_ln2, apply_bv, apply_bqk, reps)
    if key in _CACHE:
        return _CACHE[key]

    f32 = mybir.dt.float32
    bf16 = mybir.dt.bfloat16
    f8 = mybir.dt.float8e4
    AF = mybir.ActivationFunctionType

    nc = bacc.Bacc("TRN2", target_bir_lowering=False, debug=False, num_devices=NC)

    xtf_d = nc.declare_dram_parameter("xtf", [C, T], bf16, isOutput=False)
    xTq_d = nc.declare_dram_parameter("xTq", [C, R], f32, isOutput=False)
    masks_d = nc.declare_dram_parameter("masks", [P, 8, P], bf16, isOutput=False)
    ones_d = nc.declare_dram_parameter("ones", [P, P], bf16, isOutput=False)
    ind_d = nc.declare_dram_parameter("ind", [2, P], bf16, isOutput=False)
    wattn_d = nc.declare_dram_parameter("wattn", [C, C], bf16, isOutput=False)  # Q cols
    wk8_d = nc.declare_dram_parameter("wk8", [HD, CT, 2, C], f8, isOutput=False)
    wv8_d = nc.declare_dram_parameter("wv8", [HD, CT, 2, C], f8, isOutput=False)
    wproj_d = nc.declare_dram_parameter("wproj", [C, C], bf16, isOutput=False)
    wfc_d = nc.declare_dram_parameter("wfc", [C, 4 * C], bf16, isOutput=False)
    wfc2_d = nc.declare_dram_parameter("wfc2", [4 * C, C], bf16, isOutput=False)
    bqk_d = nc.declare_dram_parameter("bqk", [P, 2 * CT], f32, isOutput=False)
    bproj_d = nc.declare_dram_parameter("bproj", [P, CT], f32, isOutput=False)
    bfc_d = nc.declare_dram_parameter("bfc", [P, HT], f32, isOutput=False)
    bfc2_d = nc.declare_dram_parameter("bfc2", [P, CT], f32, isOutput=False)
    if apply_bv:
        bv_d = nc.declare_dram_parameter("bv", [P, C], f32, isOutput=False)
    if apply_ln1:
        ln1w_d = nc.declare_dram_parameter("ln1w", [P, CT], f32, isOutput=False)
        ln1b_d = nc.declare_dram_parameter("ln1b", [P, CT], f32, isOutput=False)
    if apply_ln2:
        ln2w_d = nc.declare_dram_parameter("ln2w", [P, CT], f32, isOutput=False)
        ln2b_d = nc.declare_dram_parameter("ln2b", [P, CT], f32, isOutput=False)
    outT_d = nc.declare_dram_parameter("outT", [C, R], f32, isOutput=True)

    with tile.TileContext(nc) as tc:
        with (
            tc.tile_pool(name="const", bufs=1) as const,
            tc.tile_pool(name="mid", bufs=1) as mid,
        ):
            ones_sb = const.tile([P, P], bf16)
            nc.sync.dma_start(ones_sb[:], ones_d[:])
            ind_sb = const.tile([2, P], bf16)
            nc.sync.dma_start(ind_sb[:], ind_d[:])
            masks_sb = const.tile([P, 8, P], bf16)
            nc.gpsimd.dma_start(masks_sb[:], masks_d[:])
            bqk_sb = const.tile([P, 2 * CT], f32)
            nc.sync.dma_start(bqk_sb[:], bqk_d[:])
            bproj_sb = const.tile([P, CT], f32)
            nc.gpsimd.dma_start(bproj_sb[:], bproj_d[:])
            bfc_sb = const.tile([P, HT], f32)
            nc.gpsimd.dma_start(bfc_sb[:], bfc_d[:])
            bfc2_sb = const.tile([P, CT], f32)
            nc.gpsimd.dma_start(bfc2_sb[:], bfc2_d[:])
            if apply_bv:
                bv_sb = const.tile([P, C], f32)
                nc.sync.dma_start(bv_sb[:], bv_d[:])
            ln1w_sb = ln1b_sb = ln2w_sb = ln2b_sb = None
            if apply_ln1:
                ln1w_sb = const.tile([P, CT], f32)
                ln1b_sb = const.tile([P, CT], f32)
                nc.sync.dma_start(ln1w_sb[:], ln1w_d[:])
                nc.sync.dma_start(ln1b_sb[:], ln1b_d[:])
            if apply_ln2:
                ln2w_sb = const.tile([P, CT], f32)
                ln2b_sb = const.tile([P, CT], f32)
                nc.sync.dma_start(ln2w_sb[:], ln2w_d[:])
                nc.sync.dma_start(ln2b_sb[:], ln2b_d[:])
            xTq_sb = const.tile([P, CT, R], f32)
            nc.sync.dma_start(xTq_sb[:], xTq_d.rearrange("(o p) t -> p o t", p=P))

            # mid-lifetime tiles
            # Q^T, fp8 DoubleRow-packed along hd for the S matmuls:
            # head h lives on partitions 32*(h%3) : +32 (matmul operands must
            # base at 0/32/64), group g=h//3, hd split as 32*j + p (j free).
            q8 = mid.tile([P, H // 3, 2, R], f8)
            ynorm_sb = mid.tile([P, CT, R], bf16)  # normalized attn out (y^T)
            # V resident, natural [token, feature] layout, 65 cols per head:
            # col 0 = ones (row-sum accumulator lane), cols 1:65 = V features.
            v_res = mid.tile([P, KT, H, 65], bf16)

            for _rep in range(reps):
                # ---------------- Phase A: LN1 + Q + full K/V ----------------
                ktp_cm = tc.tile_pool(name="ktp", bufs=1)
                ktp = ktp_cm.__enter__()
                # resident K^T, fp8 DoubleRow-packed along hd (same head
                # placement as q8)
                kt8 = ktp.tile([P, H // 3, 2, T], f8)
                with (
                    tc.tile_pool(name="qkvp", bufs=1) as qkvp,
                    tc.tile_pool(name="lnp", bufs=1) as lnp,
                    tc.tile_pool(name="chkp", bufs=2) as chkp,
                    tc.tile_pool(name="ps_ln", bufs=2, space="PSUM") as ps_ln,
                    tc.tile_pool(name="ps_qk", bufs=2, space="PSUM") as ps_qk,
                    tc.tile_pool(name="ps_v", bufs=2, space="PSUM") as ps_v,
                ):
                    # ones column of v_res (written once, read by every AV matmul)
                    nc.gpsimd.tensor_copy(
                        v_res[:, :, :, 0:1],
                        ones_sb[:, 0:1].to_broadcast((P, KT, H, 1)),
                    )
                    wattn_sb = qkvp.tile([P, CT, C], bf16)
                    wk8_sb = qkvp.tile([HD, CT, 2, C], f8)
                    nc.gpsimd.dma_start(wk8_sb[:], wk8_d[:])
                    wv8_sb = qkvp.tile([HD, CT, 2, C], f8)
                    nc.gpsimd.dma_start(wv8_sb[:], wv8_d[:])
                    wattn_r = wattn_d.rearrange("(o p) f -> p o f", p=P)
                    for k in range(CT):
                        nc.sync.dma_start(wattn_sb[:, k, :], wattn_r[:, k, :])

                    # Q^T for own (interleaved) query rows — first, so attention
                    # can begin as soon as the early K/V slabs land.
                    # wattn/wk8 columns are host-permuted so psum partition
                    # p^ = 64j + 32e + q holds feature 64e + 32j + q of the
                    # head pair: the two j-halves are then contiguous 64-
                    # partition blocks, repacked to DoubleRow with 2 DMAs.
                    xln_q = qkvp.tile([P, CT, R], bf16)
                    _ln_transposed(nc, tc, lnp, ps_ln, xTq_sb, xln_q, ones_sb,
                                   ln1w_sb, ln1b_sb, apply_ln1)
                    for f in range(CT):
                        ps = ps_qk.tile([P, R], f32, tag="qk_ps")
                        for k in range(CT):
                            nc.tensor.matmul(
                                ps[:], wattn_sb[:, k, P * f : P * (f + 1)],
                                xln_q[:, k, :], start=(k == 0), stop=(k == CT - 1),
                            )
                        qst = chkp.tile([P, R], f8, tag="qst", name="qst")
                        nc.vector.tensor_scalar(
                            qst[:], ps[:], bqk_sb[:, f : f + 1], None,
                            mybir.AluOpType.add,
                        )
                        for e in range(2):
                            h = 2 * f + e
                            for j in range(2):
                                nc.scalar.dma_start(
                                    q8[32 * (h % 3) : 32 * (h % 3) + 32, h // 3, j, :],
                                    qst[64 * j + 32 * e : 64 * j + 32 * e + 32, :],
                                )

                    xtf_r = xtf_d.rearrange("(o p) t -> p o t", p=P)
                    for s in range(NC):
                        xv = chkp.tile([P, CT, R], bf16, tag="xv", name="xv")
                        xeng = nc.sync if s % 2 == 0 else nc.gpsimd
                        xeng.dma_start(xv[:], xtf_r[:, :, R * s : R * (s + 1)])
                        # LN output written directly as fp8 (feeds only the
                        # fp8 K/V DoubleRow matmuls)
                        xln8 = chkp.tile([P, CT, R], f8, tag="xln8", name="xln8")
                        _ln_transposed(nc, tc, lnp, ps_ln, xv, xln8, ones_sb,
                                       ln1w_sb, ln1b_sb, apply_ln1, x_is_bf16=True)
                        # repack to DoubleRow layout [64, CT, 2, R]:
                        # contraction row c = j*64 + ki.
                        xp8 = chkp.tile([HD, CT, 2, R], f8, tag="xp8", name="xp8")
                        nc.scalar.dma_start(xp8[:, :, 0, :], xln8[0:HD, :, :])
                        nc.scalar.dma_start(xp8[:, :, 1, :], xln8[HD:P, :, :])

                        # K^T slab -> fp8 stage -> DoubleRow repack DMAs
                        for f in range(CT):
                            ps = ps_qk.tile([P, R], f32, tag="qk_ps")
                            for k in range(CT):
                                nc.tensor.matmul(
                                    ps[:], wk8_sb[:, k, :, P * f : P * (f + 1)],
                                    xp8[:, k, :, :], start=(k == 0), stop=(k == CT - 1),
                                    perf_mode=mybir.MatmulPerfMode.DoubleRow,
                                )
                            kst = chkp.tile([P, R], f8, tag="kst", name="kst")
                            keng = nc.vector if (f + s) % 2 == 0 else nc.gpsimd
                            if apply_bqk:
                                keng.tensor_scalar(
                                    kst[:], ps[:],
                                    bqk_sb[:, CT + f : CT + f + 1], None,
                                    mybir.AluOpType.add,
                                )
                            else:
                                keng.tensor_copy(kst[:], ps[:])
                            rot = [nc.sync, nc.scalar, nc.gpsimd, nc.sync]
                            for e in range(2):
                                h = 2 * f + e
                                for j in range(2):
                                    rot[(2 * e + j + f) % 4].dma_start(
                                        kt8[32 * (h % 3) : 32 * (h % 3) + 32, h // 3, j,
                                            R * s : R * (s + 1)],
                                        kst[64 * j + 32 * e : 64 * j + 32 * e + 32, :],
                                    )

                        # V slab -> natural [token, feature] layout, fp8
                        # DoubleRow matmuls, evac strided into the 65-col
                        # per-head groups of v_res
                        for t in range(QT):
                            kt = QT * s + t
                            for hh in range(2):
                                ps = ps_v.tile([P, 384], f32, tag="v_ps")
                                for k in range(CT):
                                    nc.tensor.matmul(
                                        ps[:], xp8[:, k, :, P * t : P * (t + 1)],
                                        wv8_sb[:, k, :, 384 * hh : 384 * (hh + 1)],
                                        start=(k == 0), stop=(k == CT - 1),
                                        perf_mode=mybir.MatmulPerfMode.DoubleRow,
                                    )
                                dst = v_res[:, kt, 6 * hh : 6 * (hh + 1), 1:65]
                                psv = ps[:].rearrange("p (h f) -> p h f", h=6)
                                veng = nc.vector if (t + hh) % 2 == 0 else nc.gpsimd
                                if apply_bv:
                                    bvv = bv_sb[:, 384 * hh : 384 * (hh + 1)].rearrange(
                                        "p (h f) -> p h f", h=6
                                    )
                                    veng.tensor_add(dst, psv, bvv)
                                else:
                                    veng.tensor_copy(dst, psv)

                # ---------------- Phase B: attention ----------------
                with (
                    tc.tile_pool(name="pp", bufs=2) as pp,
                    tc.tile_pool(name="normp", bufs=2) as normp,
                    tc.tile_pool(name="ps_s", bufs=2, space="PSUM") as ps_s,
                    tc.tile_pool(name="ps_y0", bufs=1, space="PSUM") as ps_y0,
                    tc.tile_pool(name="ps_y1", bufs=1, space="PSUM") as ps_y1,
                    tc.tile_pool(name="ps_bc", bufs=2, space="PSUM") as ps_bc,
                ):
                    for pr in range(PAIRS):
                        y0_ps = ps_y0.tile([P, R], f32, tag="y0")
                        y1_ps = ps_y1.tile([P, R], f32, tag="y1")
                        for m in range(4):  # bands of 8 key-tiles
                            N = P * (4 - m)
                            # p_band is h-major: [P, head-of-pair, ktile-in-band, R]
                            p_band = pp.tile([P, 2, 8, R], bf16, tag="p")
                            # exp-group size: G*N == 512 for bands 2-3, so each
                            # h-plane of the [P, 2, 512] psum tile (1 bank each,
                            # bank-aligned) holds G k-tiles' scores.
                            G = (1, 1, 2, 4)[m]
                            for g in range(8 // G):
                                s_ps = ps_s.tile([P, 2, R], f32, tag="s", name="s_ps")
                                for dg in range(G):
                                    d = g * G + dg
                                    k = 8 * m + d
                                    for e in range(2):
                                        h = 2 * pr + e
                                        hb = 32 * (h % 3)
                                        hg = h // 3
                                        nc.tensor.matmul(
                                            s_ps[:, e, dg * N : (dg + 1) * N],
                                            kt8[hb : hb + 32, hg, :, P * k : P * (k + 1)],
                                            q8[hb : hb + 32, hg, :, 0:N],
                                            perf_mode=mybir.MatmulPerfMode.DoubleRow,
                                            skip_group_check=True,
                                        )
                                if G == 1:
                                    nc.scalar.activation(
                                        p_band[:, :, g, :N], s_ps[:, :, :N],
                                        AF.Exp, scale=0.125,
                                    )
                                else:
                                    nc.scalar.activation(
                                        p_band[:, :, g * G : (g + 1) * G, :N],
                                        s_ps[:].rearrange("p h (a n) -> p h a n", n=N),
                                        AF.Exp, scale=0.125,
                                    )
                            for d in range(8):
                                k = 8 * m + d
                                # causal mask on this k-tile's diagonal col-group
                                meng = nc.gpsimd if d % 3 == 1 else nc.vector
                                meng.tensor_mul(
                                    p_band[:, :, d, N - P : N],
                                    p_band[:, :, d, N - P : N],
                                    masks_sb[:, d : d + 1, :].to_broadcast((P, 2, P)),
                                )
                                # AV + row-sum in one matmul per head: V's
                                # ones-column accumulates the row-sum into
                                # output partition 0
                                nc.tensor.matmul(
                                    y0_ps[0:65, 0:N], v_res[:, k, 2 * pr, :],
                                    p_band[:, 0, d, :N],
                                    start=(k == 0), stop=(k == KT - 1),
                                    skip_group_check=True,
                                )
                                nc.tensor.matmul(
                                    y1_ps[0:65, 0:N], v_res[:, k, 2 * pr + 1, :],
                                    p_band[:, 1, d, :N],
                                    start=(k == 0), stop=(k == KT - 1),
                                    skip_group_check=True,
                                )
                        # normalization: recip of the two row-sum rows, then
                        # 1-row ones-matmuls broadcast them across partitions
                        recip0 = normp.tile([1, R], bf16, tag="recip0")
                        recip1 = normp.tile([1, R], bf16, tag="recip1")
                        with nc.allow_low_precision(reason="softmax denom recip to bf16"):
                            nc.vector.reciprocal(recip0[:], y0_ps[0:1, :])
                            nc.vector.reciprocal(recip1[:], y1_ps[0:1, :])
                        bc_ps = ps_bc.tile([P, R], f32, tag="bc")
                        nc.tensor.matmul(bc_ps[0:HD, :], ones_sb[0:1, 0:HD], recip0[:],
                                         skip_group_check=True)
                        nc.tensor.matmul(bc_ps[HD:P, :], ones_sb[0:1, 0:HD], recip1[:],
                                         skip_group_check=True)
                        nc.vector.tensor_mul(
                            ynorm_sb[0:HD, pr, :], y0_ps[1:65, :], bc_ps[0:HD, :]
                        )
                        nc.vector.tensor_mul(
                            ynorm_sb[HD:P, pr, :], y1_ps[1:65, :], bc_ps[HD:P, :]
                        )

                ktp_cm.__exit__(None, None, None)

                # ---------------- Phase C: proj + LN2 + MLP + out ----------------
                with (
                    tc.tile_pool(name="mlpp", bufs=1) as mlpp,
                    tc.tile_pool(name="lnp2", bufs=1) as lnp2,
                ):
                    # weight loads split across queues (sync/gpsimd/vector/
                    # scalar) so no single 14us DMA gates the MLP
                    wproj_sb = mlpp.tile([P, CT, C], bf16)
                    wproj_r = wproj_d.rearrange("(o p) f -> p o f", p=P)
                    nc.sync.dma_start(wproj_sb[:, 0:3, :], wproj_r[:, 0:3, :])
                    nc.gpsimd.dma_start(wproj_sb[:, 3:6, :], wproj_r[:, 3:6, :])
                    wfc_sb = mlpp.tile([P, CT, 4 * C], bf16)
                    wfc_r = wfc_d.rearrange("(o p) f -> p o f", p=P)
                    dengs = [nc.sync, nc.gpsimd, nc.scalar, nc.gpsimd]
                    for ch in range(4):
                        dengs[ch % 4].dma_start(
                            wfc_sb[:, :, C * ch : C * (ch + 1)],
                            wfc_r[:, :, C * ch : C * (ch + 1)],
                        )
                    z_sb = mlpp.tile([P, CT, R], f32)      # residual stream x+attn
                    xln2_sb = mlpp.tile([P, CT, R], bf16)
                    wfc2_sb = mlpp.tile([P, HT, C], bf16)
                    wfc2_r = wfc2_d.rearrange("(o p) f -> p o f", p=P)
                    for ch in range(4):
                        dengs[ch % 4].dma_start(
                            wfc2_sb[:, CT * ch : CT * (ch + 1), :],
                            wfc2_r[:, CT * ch : CT * (ch + 1), :],
                        )
                    with (
                        tc.tile_pool(name="ps_proj", bufs=2, space="PSUM") as ps_proj,
                        tc.tile_pool(name="ps_ln2", bufs=1, space="PSUM") as ps_ln2,
                    ):
                        for f in range(CT):
                            ps = ps_proj.tile([P, R], f32, tag="proj")
                            for k in range(CT):
                                nc.tensor.matmul(
                                    ps[:], wproj_sb[:, k, P * f : P * (f + 1)],
                                    ynorm_sb[:, k, :], start=(k == 0), stop=(k == CT - 1),
                                )
                            # z = (proj + b_proj) + x
                            nc.vector.scalar_tensor_tensor(
                                z_sb[:, f, :], ps[:], bproj_sb[:, f : f + 1], xTq_sb[:, f, :],
                                mybir.AluOpType.add, mybir.AluOpType.add,
                            )
                        _ln_transposed(nc, tc, lnp2, ps_ln2, z_sb, xln2_sb, ones_sb,
                                       ln2w_sb, ln2b_sb, apply_ln2)

                    h_sb = mlpp.tile([P, CT, R], bf16)
                    with (
                        tc.tile_pool(name="ps_fc1", bufs=2, space="PSUM") as ps_fc1,
                        tc.tile_pool(name="ps_o", bufs=1, space="PSUM") as ps_o,
                    ):
                        o_ps = [ps_o.tile([P, R], f32, tag=f"o{f}", name=f"o_ps{f}") for f in range(CT)]
                        for chunk in range(4):
                            for hf in range(CT):
                                hh = CT * chunk + hf
                                ps = ps_fc1.tile([P, R], f32, tag="fc1")
                                for k in range(CT):
                                    nc.tensor.matmul(
                                        ps[:], wfc_sb[:, k, P * hh : P * (hh + 1)],
                                        xln2_sb[:, k, :], start=(k == 0), stop=(k == CT - 1),
                                    )
                                nc.scalar.activation(
                                    h_sb[:, hf, :], ps[:], AF.Gelu, bias=bfc_sb[:, hh : hh + 1]
                                )
                                for f in range(CT):
                                    nc.tensor.matmul(
                                        o_ps[f][:], wfc2_sb[:, hh, P * f : P * (f + 1)],
                                        h_sb[:, hf, :], start=(hh == 0), stop=(hh == HT - 1),
                                    )
                        for f in range(CT):
                            # out = (o + b_fc2) + z, in place over z
                            nc.vector.scalar_tensor_tensor(
                                z_sb[:, f, :], o_ps[f][:], bfc2_sb[:, f : f + 1], z_sb[:, f, :],
                                mybir.AluOpType.add, mybir.AluOpType.add,
                            )
                            nc.sync.dma_start(outT_d[P * f : P * (f + 1), :], z_sb[:, f, :])

    nc.compile()
    _CACHE[key] = nc
    return nc


def _query_tokens(c):
    """Token ids owned by core c, in on-chip column order (j desc, i asc)."""
    return np.concatenate([1024 * j + 8 * np.arange(P) + c for j in (3, 2, 1, 0)])


def kernel(x, ln1_w, ln1_b, W_attn, b_attn, W_proj, b_proj,
           ln2_w, ln2_b, W_fc, b_fc, W_fc2, b_fc2):
    x = np.asarray(x, np.float32)
    ln1_w = np.asarray(ln1_w, np.float32)
    ln1_b = np.asarray(ln1_b, np.float32)
    W_attn = np.asarray(W_attn, np.float32)
    b_attn = np.asarray(b_attn, np.float32)
    W_proj = np.asarray(W_proj, np.float32)
    b_proj = np.asarray(b_proj, np.float32)
    ln2_w = np.asarray(ln2_w, np.float32)
    ln2_b = np.asarray(ln2_b, np.float32)
    W_fc = np.asarray(W_fc, np.float32)
    b_fc = np.asarray(b_fc, np.float32)
    W_fc2 = np.asarray(W_fc2, np.float32)
    b_fc2 = np.asarray(b_fc2, np.float32)

    apply_ln1 = not (np.all(ln1_w == 1.0) and np.all(ln1_b == 0.0))
    apply_ln2 = not (np.all(ln2_w == 1.0) and np.all(ln2_b == 0.0))
    apply_bv = bool(np.any(b_attn[2 * C :] != 0.0))
    apply_bqk = bool(np.any(b_attn[: 2 * C] != 0.0))

    nc = _build(apply_ln1, apply_ln2, apply_bv, apply_bqk)

    xf = x[0]  # [T, C]
    # Q/K psum partition permutation: psum partition p^ = 64j + 32e + q
    # holds pair-local feature 64e + 32j + q (e = head-in-pair, hd = 32j+q),
    # so the two j-halves are contiguous 64-partition blocks for the
    # DoubleRow repack DMAs.
    ph = np.arange(P)
    PERM = 64 * ((ph % 64) // 32) + 32 * (ph // 64) + ph % 32
    wq = W_attn[:, :C].reshape(C, CT, P)[:, :, PERM].reshape(C, C)
    wattn_b = np.ascontiguousarray(wq).astype(BF16)
    # DoubleRow packing: [ki, kk, j, f] = W[kk*128 + j*64 + ki, col0 + f]
    wkp = W_attn[:, C : 2 * C].reshape(C, CT, P)[:, :, PERM].reshape(C, C)
    wk = wkp.reshape(CT, 2, HD, C)  # [kk, j, ki, f]
    wk8 = np.ascontiguousarray(wk.transpose(2, 0, 1, 3)).astype(F8)
    wv = W_attn[:, 2 * C :].reshape(CT, 2, HD, C)
    wv8 = np.ascontiguousarray(wv.transpose(2, 0, 1, 3)).astype(F8)
    wproj_b = W_proj.astype(BF16)
    wfc_b = W_fc.astype(BF16)
    wfc2_b = W_fc2.astype(BF16)
    bqk = np.ascontiguousarray(b_attn[: 2 * C].reshape(2 * CT, P)[:, PERM].T)
    bproj = np.ascontiguousarray(b_proj.reshape(CT, P).T)
    bfc = np.ascontiguousarray(b_fc.reshape(HT, P).T)
    bfc2 = np.ascontiguousarray(b_fc2.reshape(CT, P).T)
    ones = np.ones((P, P), BF16)
    ind = np.zeros((2, P), BF16)
    ind[0, 0:HD] = 1.0
    ind[1, HD:P] = 1.0

    xtf = np.ascontiguousarray(xf.T.astype(BF16))
    in_maps = []
    qtok = []
    for c in range(NC):
        qt = _query_tokens(c)
        qtok.append(qt)
        xTq = np.ascontiguousarray(xf[qt, :].T)
        kk = np.arange(P)[:, None, None]
        dd = np.arange(8)[None, :, None]
        ii = np.arange(P)[None, None, :]
        masks = ((8 * ii + c - 128 * dd - kk) >= 0).astype(BF16)
        m = {
            "xtf": xtf, "xTq": xTq, "masks": masks, "ones": ones, "ind": ind,
            "wk8": wk8, "wv8": wv8,
            "wattn": wattn_b, "wproj": wproj_b, "wfc": wfc_b, "wfc2": wfc2_b,
            "bqk": bqk, "bproj": bproj, "bfc": bfc, "bfc2": bfc2,
        }
        if apply_bv:
            m["bv"] = np.ascontiguousarray(np.broadcast_to(b_attn[2 * C :], (P, C)))
        if apply_ln1:
            m["ln1w"] = np.ascontiguousarray(ln1_w.reshape(CT, P).T)
            m["ln1b"] = np.ascontiguousarray(ln1_b.reshape(CT, P).T)
        if apply_ln2:
            m["ln2w"] = np.ascontiguousarray(ln2_w.reshape(CT, P).T)
            m["ln2b"] = np.ascontiguousarray(ln2_b.reshape(CT, P).T)
        in_maps.append(m)

    res = run_bass_kernel_spmd(nc, in_maps, list(range(NC)))

    out = np.empty((T, C), np.float32)
    for c in range(NC):
        out[qtok[c], :] = res.results[c]["outT"].T
    return out[None, :, :]
